# revision 55
# baseline (speedup 1.0000x reference)
"""Baichuan sliding-window GQA attention block on 8 trn2 NeuronCores.

Sharding: data-parallel over batch (2) x tensor-parallel over heads (4).
Core c handles batch b=c//4, head group g=c%4 (q heads 4g..4g+3, kv heads
2g..2g+1). Each core computes qkv projection, RoPE, 2-tap causal conv,
windowed attention and a row-sharded o_proj partial; the host sums the 4
partials per batch.

v3: software-pipelined 512-token super-chunk loop. Iteration t emits,
interleaved at instruction level: qkv matmuls for super-chunk t,
attention tile-steps for super-chunk t-1 (two 256-token q-chunks), and
o_proj for super-chunk t-2 — so the PE always has independent GEMM work
between an attention tile's scores (PE) -> exp (ACT) -> mask/fold (DVE)
-> pv (PE) chain. Every PSUM accumulation group covers exactly one full
2KB bank (a matmul group start zeroes its entire bank on TRN2, so banks
cannot be shared between concurrent groups). bf16 storage everywhere
(PE 1 cyc/row, DMA halved, DVE 2-4x modes). Conv w1 is folded into
W_k/W_v host-side (rope is linear) making each conv one
scalar_tensor_tensor per head; masking is multiplicative post-exp;
softmax denominators come from bf16 DVE tile-folds + one [1,512]
ones-matmul per (q-chunk, kv head); V is conv'd in [d, tok] layout then
PE-transposed (bf16) into [tok, d] tiles for the PV matmul.

Layouts (per core, on-chip):
  qpair[i] [128d, 2*S]   roped Q, the 2 q-heads sharing kv head i
  kct[i][j] [128d, 128]  roped+conv'd K tile (w1k folded into weights)
  vt[i][j] [128tok,128d] conv'd V tile for token block j (w1v in weights)
  scoresT[k,q] = sum_d kT[d,k] qT[d,q]; outT[d,q] = sum_k v[k,d] pT[k,q]
"""

import numpy as np
import ml_dtypes

B, S, H = 2, 2048, 2048
NH, NKV, HD = 16, 8, 128
WINDOW = 1024
THETA = 100000.0
TP = 4                      # tensor-parallel ways (head groups)
QH = NH // TP               # 4 q heads per core
KVH = NKV // TP             # 2 kv heads per core
NCORES = 8
SCALE = 1.0 / float(np.sqrt(HD))

NT = S // 256               # 8 attention q-chunks of 256
NSC = S // 512              # 4 super-chunks of 512
NK = H // 128               # 16 contraction tiles

_CACHE = {}


def _build_program():
    import concourse.bacc as bacc
    import concourse.mybir as mybir
    import concourse.tile as tile

    f32 = mybir.dt.float32
    bf16 = mybir.dt.bfloat16
    Exp = mybir.ActivationFunctionType.Exp
    mult = mybir.AluOpType.mult
    add = mybir.AluOpType.add

    nc = bacc.Bacc("TRN2", target_bir_lowering=False, debug=False,
                   enable_asserts=False, num_devices=NCORES)

    hT_d = nc.dram_tensor("hT", [H, S], bf16, kind="ExternalInput")
    wpk_d = nc.dram_tensor("wpk", [H, 1024], bf16, kind="ExternalInput")
    wo_d = nc.dram_tensor("wo", [QH * HD, H], bf16, kind="ExternalInput")
    # per super-chunk: [cos dup x2 head-cols (1024) | +-sin dup (1024)]
    csn_d = nc.dram_tensor("csn", [128, NSC * 2048], bf16,
                           kind="ExternalInput")
    cw_d = nc.dram_tensor("cw", [128, 4], f32, kind="ExternalInput")
    # multiplicative bf16 masks: [m1024h 256 | m896 512 | m0 512 | mm128h 256]
    msk_d = nc.dram_tensor("msk", [128, 1536], bf16, kind="ExternalInput")
    one_d = nc.dram_tensor("one", [128, 1], bf16, kind="ExternalInput")
    eye_d = nc.dram_tensor("eye", [128, 128], bf16, kind="ExternalInput")
    yT_d = nc.dram_tensor("yT", [H, S], bf16, kind="ExternalOutput")

    with tile.TileContext(nc) as tc:
        with (
            tc.tile_pool(name="const", bufs=1) as cp,
            tc.tile_pool(name="wts", bufs=1) as wp,
            tc.tile_pool(name="persist", bufs=1) as pp,
            tc.tile_pool(name="ht", bufs=2) as bht,
            tc.tile_pool(name="roll", bufs=2) as rl,
            tc.tile_pool(name="ebuf", bufs=2) as eb,
            tc.tile_pool(name="pb", bufs=6) as pbp,
            tc.tile_pool(name="accp", bufs=3) as accp,
            tc.tile_pool(name="accf", bufs=2) as accfp,
            tc.tile_pool(name="rbp", bufs=3) as rbp,
            tc.tile_pool(name="ybp", bufs=3) as ybp,
            tc.tile_pool(name="mm", bufs=4, space="PSUM") as mm,
            tc.tile_pool(name="scp", bufs=2, space="PSUM") as scp,
            tc.tile_pool(name="pvp", bufs=2, space="PSUM") as pvp,
        ):
            # --- weight + first super-chunk loads, interleaved so the
            # DMA-bound startup overlaps: wf pairs alternate ht0 quarters
            wfc = [wp.tile([128, 2048], bf16, tag=f"wfc{p}", name=f"wfc{p}")
                   for p in range(NK // 2)]

            def wfs(k, lo, hi):
                o = (k % 2) * 1024
                return wfc[k // 2][:, o + lo:o + hi]

            htile = bht.tile([128, NK * 512], bf16, tag="htc", name="htc")
            htq_after = {0: 0, 1: 2, 2: 4, 3: 6}
            for p in range(NK // 2):
                nc.sync.dma_start(
                    out=wfc[p][:].rearrange("q (k c) -> q k c", k=2),
                    in_=wpk_d[p * 256:(p + 1) * 256, :].rearrange(
                        "(k q) c -> q k c", k=2))
                for q, after in htq_after.items():
                    if after == p:
                        nc.sync.dma_start(
                            out=htile[:, q * 2048:(q + 1) * 2048].rearrange(
                                "w (k s) -> w k s", k=4),
                            in_=hT_d[q * 512:(q + 1) * 512, 0:512].rearrange(
                                "(k w) s -> w k s", k=4))

            def load_cssn(sc):
                csn = rl.tile([128, 2048], bf16, tag="csr", name="csr")
                nc.sync.dma_start(out=csn[:],
                                  in_=csn_d[:, sc * 2048:(sc + 1) * 2048])
                return csn

            cssn = load_cssn(0)
            htile1 = bht.tile([128, NK * 512], bf16, tag="htc",
                              name="htc")
            nc.sync.dma_start(
                out=htile1[:].rearrange("p (k s) -> p k s", k=NK),
                in_=hT_d[:, 512:1024].rearrange("(k p) s -> p k s", k=NK))
            cw_sb = cp.tile([128, 4], f32, tag="cw", name="cw")
            msk_sb = cp.tile([128, 1536], bf16, tag="msk", name="msk")
            one_sb = cp.tile([128, 1], bf16, tag="one", name="one")
            eye_sb = cp.tile([128, 128], bf16, tag="eye", name="eye")
            nc.sync.dma_start(out=cw_sb[:], in_=cw_d[:, :])
            nc.sync.dma_start(out=msk_sb[:], in_=msk_d[:, :])
            nc.sync.dma_start(out=one_sb[:], in_=one_d[:, :])
            nc.sync.dma_start(out=eye_sb[:], in_=eye_d[:, :])
            wo_sb = wp.tile([128, QH * H], bf16, tag="wo", name="wo")

            # --- persistent activations (bf16)
            qpair = [pp.tile([128, 2 * S], bf16, tag=f"qp{i}", name=f"qp{i}")
                     for i in range(KVH)]
            kct = [[pp.tile([128, 128], bf16, tag=f"kc{i}_{j}",
                            name=f"kc{i}_{j}") for j in range(NK)]
                   for i in range(KVH)]
            vt = [[pp.tile([128, 128], bf16, tag=f"vt{i}_{j}",
                           name=f"vt{i}_{j}") for j in range(NK)]
                  for i in range(KVH)]
            attn = [pp.tile([128, S], bf16, tag=f"at{h}", name=f"at{h}")
                    for h in range(QH)]

            M1024H, M896, M0, MM128H = 0, 256, 768, 1280

            def make_sum_step(st, qc, i):
                """Lazy: reads st['acc']/st['ps_o'] at emission time."""
                def emit():
                    ps_s = scp.tile([128, 512], f32, tag="sc", name="sc")
                    nc.tensor.matmul(ps_s[0:1, :], one_sb[:], st["acc"][:],
                                     start=True, stop=True)
                    rsum = eb.tile([1, 512], f32, tag="rs", name="rs")
                    nc.vector.reciprocal(rsum[:], ps_s[0:1, :])
                    rb = rbp.tile([128, 512], f32, tag="rb", name="rb")
                    nc.gpsimd.partition_broadcast(rb[:], rsum[:])
                    nc.vector.tensor_mul(attn[2 * i][:, qc:qc + 256],
                                         st["ps_o"][:, 0:256], rb[:, 0:256])
                    nc.vector.tensor_mul(attn[2 * i + 1][:, qc:qc + 256],
                                         st["ps_o"][:, 256:512],
                                         rb[:, 256:512])
                return emit

            def attention_steps(qi, carry_in, last=False):
                """Step closures for q-chunk qi. carry_in: the previous
                head's softmax-denominator step, woven in near the start of
                head 0's stream. Returns (steps, carry_out)."""
                steps = []
                carry = carry_in
                qc = qi * 256
                for i in range(KVH):
                    st = {"acc": None, "ps_o": None, "po3": None}
                    q2 = qpair[i][:].rearrange("p (h s) -> p h s", h=2)
                    jstart = max(0, qc // 128 - 8)
                    jend = qc // 128 + 1
                    fulls = [j for j in range(jstart, jend)
                             if qc - j * 128 != 1024]
                    halves = ([j for j in range(jstart, jend)
                               if qc - j * 128 == 1024] + [jend])
                    tiles = ([("f", j) for j in fulls] +
                             [("h", j) for j in halves])
                    ntile = len(tiles)

                    def a_step(kind, j, idx, st=st, i=i, qc=qc, q2=q2,
                               ntile=ntile):
                        delta = qc - j * 128
                        ps_sc = scp.tile([128, 512], f32, tag="sc", name="sc")
                        if kind == "f":
                            nc.tensor.matmul(
                                ps_sc[:], kct[i][j][:],
                                q2[:, :, qc:qc + 256],
                                start=True, stop=True)
                            pb = pbp.tile([128, 512], bf16, tag="pb",
                                          name="pb")
                            if idx == 0 and delta not in (896, 0):
                                acc = accp.tile([128, 512], bf16, tag="acc",
                                                name="acc")
                                nc.scalar.activation(acc[:], ps_sc[:], Exp,
                                                     bias=0.0, scale=SCALE)
                                st["acc"] = acc
                                st["pb"] = acc
                                return
                            nc.scalar.activation(pb[:], ps_sc[:], Exp,
                                                 bias=0.0, scale=SCALE)
                            if delta in (896, 0):
                                moff = M896 if delta == 896 else M0
                                if idx == 0:
                                    acc = accp.tile([128, 512], bf16,
                                                    tag="acc", name="acc")
                                    nc.vector.tensor_mul(
                                        acc[:], pb[:],
                                        msk_sb[:, moff:moff + 512])
                                    st["acc"] = acc
                                    st["pb"] = acc
                                    return
                                nc.vector.tensor_mul(
                                    pb[:], pb[:],
                                    msk_sb[:, moff:moff + 512])
                            nacc = accp.tile([128, 512], bf16, tag="acc",
                                             name="acc")
                            nc.vector.tensor_add(nacc[:], pb[:],
                                                 st["acc"][:])
                            st["acc"] = nacc
                            st["pb"] = pb
                        else:
                            qoff = 0 if delta == 1024 else 128
                            moff = M1024H if delta == 1024 else MM128H
                            ps3 = ps_sc[:].rearrange("p (h q) -> p h q", h=2)
                            nc.tensor.matmul(
                                ps3[:, :, 0:128], kct[i][j][:],
                                q2[:, :, qc + qoff:qc + qoff + 128],
                                start=True, stop=True)
                            pb = pbp.tile([128, 512], bf16, tag="pb",
                                          name="pb")
                            pb3 = pb[:].rearrange("p (h q) -> p h q", h=2)
                            nc.scalar.activation(pb3[:, :, 0:128],
                                                 ps3[:, :, 0:128], Exp,
                                                 bias=0.0, scale=SCALE)
                            m3 = msk_sb[:, moff:moff + 256].rearrange(
                                "p (h q) -> p h q", h=2)
                            nc.vector.tensor_mul(pb3[:, :, 0:128],
                                                 pb3[:, :, 0:128],
                                                 m3[:, :, :])
                            last_fold = (idx == ntile - 1)
                            pool = accfp if last_fold else accp
                            tag = "accf" if last_fold else "acc"
                            nacc = pool.tile([128, 512], bf16, tag=tag,
                                             name=tag)
                            acc3 = st["acc"][:].rearrange(
                                "p (h q) -> p h q", h=2)
                            nacc3 = nacc[:].rearrange("p (h q) -> p h q", h=2)
                            nc.vector.tensor_add(
                                nacc3[:, :, qoff:qoff + 128],
                                pb3[:, :, 0:128],
                                acc3[:, :, qoff:qoff + 128])
                            oq = 128 - qoff
                            nc.vector.tensor_copy(
                                nacc3[:, :, oq:oq + 128],
                                acc3[:, :, oq:oq + 128])
                            st["acc"] = nacc
                            st["pb"] = pb

                    def b_step(kind, j, idx, st=st, i=i, ntile=ntile, qc=qc):
                        first = (idx == 0)
                        fin = (idx == ntile - 1)
                        if first:
                            st["ps_o"] = pvp.tile([128, 512], f32, tag="pv",
                                                  name="pv")
                            st["po3"] = st["ps_o"][:].rearrange(
                                "p (h q) -> p h q", h=2)
                        pb = st[f"pb{idx}"]
                        if kind == "f":
                            nc.tensor.matmul(st["ps_o"][:], vt[i][j][:],
                                             pb[:], start=first, stop=fin)
                        else:
                            delta = qc - j * 128
                            qoff = 0 if delta == 1024 else 128
                            pb3 = pb[:].rearrange("p (h q) -> p h q", h=2)
                            nc.tensor.matmul(
                                st["po3"][:, :, qoff:qoff + 128],
                                vt[i][j][:], pb3[:, :, 0:128],
                                start=False, stop=fin)

                    def make_a(kind, j, idx, a_step=a_step, st=st):
                        def f():
                            a_step(kind, j, idx)
                            st[f"pb{idx}"] = st["pb"]
                        return f

                    def make_b(kind, j, idx, b_step=b_step):
                        return lambda: b_step(kind, j, idx)

                    seq = []
                    seq.append(make_a(*tiles[0], 0))
                    if ntile > 1:
                        seq.append(make_a(*tiles[1], 1))
                    if carry is not None:
                        seq.append(carry)
                        carry = None
                    seq.append(make_b(*tiles[0], 0))
                    for idx in range(2, ntile):
                        seq.append(make_a(*tiles[idx], idx))
                        seq.append(make_b(*tiles[idx - 1], idx - 1))
                    if ntile > 1:
                        seq.append(make_b(*tiles[ntile - 1], ntile - 1))
                    sum_step = make_sum_step(st, qc, i)
                    if last and i == KVH - 1:
                        seq.append(sum_step)
                        carry = None
                    else:
                        carry = sum_step
                    steps.extend(seq)
                return steps, carry

            def oproj_units(sc, qhalf=None, alt=False):
                """16 units: 4 accumulating matmuls over [128,512] tokens
                (one full psum bank) + psum->sbuf copy + one DMA out.
                qhalf selects a 256-token half (for the tail)."""
                lo = sc * 512 + (0 if qhalf in (None, 0) else 256)
                w = 512 if qhalf is None else 256
                units = []
                for oc in range(NK):
                    def unit(oc=oc, sc=sc, lo=lo, w=w, alt=alt):
                        ps_y = mm.tile([128, 512], f32, tag="qkv", name="op")
                        for d in range(QH):
                            nc.tensor.matmul(
                                ps_y[:, 0:w],
                                wo_sb[:, d * H + oc * 128:
                                      d * H + (oc + 1) * 128],
                                attn[d][:, lo:lo + w],
                                start=(d == 0), stop=(d == QH - 1))
                        yb = ybp.tile([128, 512], bf16, tag="yb",
                                      name="yb")
                        if alt and oc % 2 == 1:
                            nc.vector.tensor_copy(yb[:, 0:w], ps_y[:, 0:w])
                        else:
                            nc.scalar.copy(yb[:, 0:w], ps_y[:, 0:w])
                        nc.sync.dma_start(
                            out=yT_d[oc * 128:(oc + 1) * 128, lo:lo + w],
                            in_=yb[:, 0:w])
                    units.append(unit)
                return units

            krprev = [None]
            vrawprev = [None]

            def rope_pair(sc, cp_, srcs, cssn_t, kr):
                """srcs: two [128,512] col tensors. cp_<2 -> qpair[cp_];
                cp_==2 -> kr. out = src*cos + swap64(src)*(+-sin)."""
                csl = cssn_t[:, 0:1024]
                snl = cssn_t[:, 1024:2048]
                if cp_ < 2:
                    dst3 = qpair[cp_][:].rearrange(
                        "p (h s) -> p h s", h=2)[:, :,
                                                 sc * 512:(sc + 1) * 512]
                else:
                    dst3 = kr[:].rearrange("p (h s) -> p h s", h=2)
                e1 = eb.tile([128, 1024], f32, tag="e1", name="e1")
                e2 = eb.tile([128, 1024], f32, tag="e2", name="e2")
                for h in range(2):
                    sl = slice(h * 512, (h + 1) * 512)
                    nc.vector.tensor_mul(e1[:, sl], srcs[h][:], csl[:, sl])
                    nc.vector.tensor_mul(e2[0:64, sl], srcs[h][64:128, :],
                                         snl[0:64, sl])
                    nc.vector.tensor_mul(e2[64:128, sl], srcs[h][0:64, :],
                                         snl[64:128, sl])
                e13 = e1[:].rearrange("p (h s) -> p h s", h=2)
                e23 = e2[:].rearrange("p (h s) -> p h s", h=2)
                nc.gpsimd.tensor_add(dst3[:, :, :], e13[:, :, :],
                                     e23[:, :, :])

            def kconv_emit(sc, kr):
                # kconv = kr + (w0k/w1k) * kr_prev-token (w1k in weights)
                for i in range(KVH):
                    r = cw_sb[:, i:i + 1]
                    b = i * 512
                    for jj in range(4):
                        j = 4 * sc + jj
                        lo = b + jj * 128
                        if jj == 0:
                            nc.vector.scalar_tensor_tensor(
                                kct[i][j][:, 1:128], kr[:, b:b + 127], r,
                                kr[:, b + 1:b + 128], mult, add)
                            if sc == 0:
                                nc.vector.tensor_copy(kct[i][j][:, 0:1],
                                                      kr[:, b:b + 1])
                            else:
                                nc.vector.scalar_tensor_tensor(
                                    kct[i][j][:, 0:1],
                                    krprev[0][:, b + 511:b + 512], r,
                                    kr[:, b:b + 1], mult, add)
                        else:
                            nc.vector.scalar_tensor_tensor(
                                kct[i][j][:, 0:128],
                                kr[:, lo - 1:lo + 127], r,
                                kr[:, lo:lo + 128], mult, add)
                krprev[0] = kr

            def vconv_pre(sc, psV):
                """psV: 2 psum tiles [128 d, 512 tok] (one per kv head).
                Copy to SBUF bf16, conv along tokens. Returns vcb."""
                vcb = rl.tile([128, 1024], bf16, tag="vcb", name="vcb")
                vraw = rl.tile([128, 1024], bf16, tag="vraw", name="vraw")
                for i in range(KVH):
                    b = i * 512
                    nc.scalar.copy(vraw[:, b:b + 512], psV[i][:])
                    r = cw_sb[:, 2 + i:3 + i]
                    nc.vector.scalar_tensor_tensor(
                        vcb[:, b + 1:b + 512], vraw[:, b:b + 511], r,
                        vraw[:, b + 1:b + 512], mult, add)
                    if sc == 0:
                        nc.vector.tensor_copy(vcb[:, b:b + 1],
                                              vraw[:, b:b + 1])
                    else:
                        nc.vector.scalar_tensor_tensor(
                            vcb[:, b:b + 1],
                            vrawprev[0][:, b + 511:b + 512], r,
                            vraw[:, b:b + 1], mult, add)
                vrawprev[0] = vraw
                return vcb

            def vconv_transposes(sc, vcb):
                for half in range(2):
                    tps = scp.tile([128, 512], f32, tag="sc", name="vtp")
                    tpb = tps[:].bitcast(bf16)
                    for u in range(4):
                        g = 4 * half + u      # global 128-block 0..7
                        i, sub = g // 4, g % 4
                        nc.tensor.transpose(
                            tpb[:, u * 128:(u + 1) * 128],
                            vcb[:, i * 512 + sub * 128:
                                i * 512 + (sub + 1) * 128],
                            eye_sb[:])
                        nc.vector.tensor_copy(vt[i][4 * sc + sub][:],
                                              tpb[:, u * 128:(u + 1) * 128])

            carry = None

            # ---------------- main software-pipelined loop ----------------
            for t in range(NSC + 1):
                if t >= 1:
                    sa, carry = attention_steps(2 * (t - 1), carry)
                    sb, carry = attention_steps(
                        2 * (t - 1) + 1, carry,
                        last=(2 * (t - 1) + 1 == NT - 1))
                    steps = sa + sb
                else:
                    steps = []
                units = (oproj_units(t - 2, alt=(t == NSC))
                         if t >= 2 else [])
                if t < NSC:
                    fill = steps
                else:
                    fill = steps[:4]
                    rest = steps[4:]
                    for u in range(max(len(units), len(rest))):
                        if u < len(units):
                            fill.append(units[u])
                        if u < len(rest):
                            fill.append(rest[u])
                si = 0

                if t < NSC:
                    if t + 1 < NSC:
                        if t == 0:
                            nhtile = htile1
                        else:
                            nhtile = bht.tile([128, NK * 512], bf16,
                                              tag="htc", name="htc")
                            nc.sync.dma_start(
                                out=nhtile[:].rearrange("p (k s) -> p k s",
                                                        k=NK),
                                in_=hT_d[:, (t + 1) * 512:(t + 2) * 512]
                                .rearrange("(k p) s -> p k s", k=NK))
                        ncssn = load_cssn(t + 1)
                    if t == 0:
                        nc.sync.dma_start(
                            out=wo_sb[:].rearrange("p (d c) -> p d c", d=QH),
                            in_=wo_d[:, :].rearrange("(d p) c -> p d c",
                                                     d=QH))

                    # --- qkv: col-outer, k-inner; one group per full bank.
                    # cols 0..3 = Q heads, 4..5 = K heads, 6..7 = V heads.
                    col_src = [None] * 8
                    nslot = 8 * NK
                    slot = 0
                    kr = rl.tile([128, 1024], bf16, tag="kr", name="kr")
                    vcb = None
                    if True:
                        for col in range(8):
                            ps = mm.tile([128, 512], f32, tag="qkv",
                                         name=f"ps{col}")
                            for k in range(NK):
                                nc.tensor.matmul(
                                    ps[:],
                                    wfs(k, col * 128, (col + 1) * 128),
                                    htile[:, k * 512:(k + 1) * 512],
                                    start=(k == 0), stop=(k == NK - 1))
                                slot += 1
                                want = slot * len(fill) * 3 // (4 * nslot)
                                while si < min(want, len(fill)):
                                    fill[si]()
                                    si += 1
                            col_src[col] = ps
                            if col == 1:
                                rope_pair(t, 0, col_src[0:2], cssn, None)
                            elif col == 3:
                                rope_pair(t, 1, col_src[2:4], cssn, None)
                            elif col == 5:
                                rope_pair(t, 2, col_src[4:6], cssn, kr)
                                kconv_emit(t, kr)
                            elif col == 7:
                                vcb = vconv_pre(t, col_src[6:8])
                    nu = len(units)
                    rest = len(fill) - si
                    for ui in range(max(nu, 1)):
                        if ui < nu:
                            units[ui]()
                        tgt = si + (rest // max(nu, 1)) if ui < nu - 1 \
                            else len(fill)
                        while si < min(tgt, len(fill)):
                            fill[si]()
                            si += 1
                    vconv_transposes(t, vcb)
                    if t + 1 < NSC:
                        htile = nhtile
                        cssn = ncssn
                else:
                    while si < len(fill):
                        fill[si]()
                        si += 1
                    for u in oproj_units(NSC - 1, alt=True):
                        u()

    nc.finalize()
    return nc


def _host_inputs(hidden, W_pack, W_o, conv_k, conv_v):
    """Per-core input maps."""
    bf16 = ml_dtypes.bfloat16
    pos = np.arange(S, dtype=np.float64)
    inv_freq = 1.0 / (THETA ** (np.arange(0, HD, 2, dtype=np.float64) / HD))
    freqs = np.outer(pos, inv_freq)                       # (S, 64)
    cos = np.cos(freqs).T.astype(np.float32)              # (64, S)
    sin = np.sin(freqs).T.astype(np.float32)
    cos = np.concatenate([cos, cos], axis=0)              # (128, S)
    sin = np.concatenate([-sin, sin], axis=0)             # sign folded in
    # per super-chunk: [cos dup x2 head-cols (1024) | +-sin dup (1024)]
    cs = np.broadcast_to(
        cos.reshape(128, NSC, 1, 512),
        (128, NSC, 2, 512)).reshape(128, NSC, 1024)
    sn = np.broadcast_to(
        sin.reshape(128, NSC, 1, 512),
        (128, NSC, 2, 512)).reshape(128, NSC, 1024)
    csn = np.concatenate([cs, sn], axis=2).reshape(128, -1)
    csn = np.ascontiguousarray(csn).astype(bf16)

    kk = np.arange(128)[:, None]
    qq = np.arange(128)[None, :]
    qq2 = np.arange(256)[None, :]

    def double(m):
        return np.concatenate([m, m], axis=1)
    m1024h = double(qq < kk)                       # [128, 256]
    m896 = double(qq2 - kk < 128)                  # [128, 512]
    m0 = double(qq2 >= kk)                         # [128, 512]
    mm128h = double(qq >= kk)                      # [128, 256]
    msk = np.concatenate([m1024h, m896, m0, mm128h],
                         axis=1).astype(bf16)      # [128, 1536]

    one = np.ones((128, 1), bf16)
    eye = np.eye(128).astype(bf16)

    in_maps = []
    for c in range(NCORES):
        b, g = c // TP, c % TP
        hT = np.ascontiguousarray(hidden[b].T).astype(bf16)
        wq = W_pack[:, g * 512:(g + 1) * 512]
        wk = W_pack[:, NH * HD + 2 * g * 128: NH * HD + (2 * g + 2) * 128]
        wv = W_pack[:, NH * HD + NKV * HD + 2 * g * 128:
                    NH * HD + NKV * HD + (2 * g + 2) * 128]
        # fold conv w1 into Wk/Wv (rope is linear; conv comes after rope)
        wk = wk.copy()
        wv = wv.copy()
        for i in range(KVH):
            wk[:, i * 128:(i + 1) * 128] *= conv_k[2 * g + i, 1]
            wv[:, i * 128:(i + 1) * 128] *= conv_v[2 * g + i, 1]
        wpk = np.ascontiguousarray(
            np.concatenate([wq, wk, wv], axis=1)).astype(bf16)
        wo = np.ascontiguousarray(
            W_o[g * 512:(g + 1) * 512, :]).astype(bf16)
        cwv = np.empty(4, np.float32)
        for i in range(KVH):
            cwv[i] = conv_k[2 * g + i, 0] / conv_k[2 * g + i, 1]
            cwv[2 + i] = conv_v[2 * g + i, 0] / conv_v[2 * g + i, 1]
        cw = np.broadcast_to(cwv, (128, 4)).copy()
        in_maps.append({
            "hT": hT, "wpk": wpk, "wo": wo, "csn": csn,
            "cw": cw, "msk": msk, "one": one, "eye": eye,
        })
    return in_maps


def run_cores(in_maps, trace=False, **kw):
    from concourse.bass_utils import run_bass_kernel_spmd
    if "nc" not in _CACHE:
        _CACHE["nc"] = _build_program()
    return run_bass_kernel_spmd(_CACHE["nc"], in_maps, list(range(NCORES)),
                                trace=trace, **kw)


def kernel(hidden, W_pack, W_o, conv_k, conv_v):
    hidden = np.asarray(hidden, np.float32)
    W_pack = np.asarray(W_pack, np.float32)
    W_o = np.asarray(W_o, np.float32)
    conv_k = np.asarray(conv_k, np.float32)
    conv_v = np.asarray(conv_v, np.float32)
    in_maps = _host_inputs(hidden, W_pack, W_o, conv_k, conv_v)
    res = run_cores(in_maps)
    out = np.zeros((B, S, H), np.float32)
    for c in range(NCORES):
        b = c // TP
        out[b] += res.results[c]["yT"].T.astype(np.float32)
    return out


# revision 56
# speedup vs baseline: 1.0006x; 1.0006x over previous
"""Baichuan sliding-window GQA attention block on 8 trn2 NeuronCores.

Sharding: data-parallel over batch (2) x tensor-parallel over heads (4).
Core c handles batch b=c//4, head group g=c%4 (q heads 4g..4g+3, kv heads
2g..2g+1). Each core computes qkv projection, RoPE, 2-tap causal conv,
windowed attention and a row-sharded o_proj partial; the host sums the 4
partials per batch.

v3: software-pipelined 512-token super-chunk loop. Iteration t emits,
interleaved at instruction level: qkv matmuls for super-chunk t,
attention tile-steps for super-chunk t-1 (two 256-token q-chunks), and
o_proj for super-chunk t-2 — so the PE always has independent GEMM work
between an attention tile's scores (PE) -> exp (ACT) -> mask/fold (DVE)
-> pv (PE) chain. Every PSUM accumulation group covers exactly one full
2KB bank (a matmul group start zeroes its entire bank on TRN2, so banks
cannot be shared between concurrent groups). bf16 storage everywhere
(PE 1 cyc/row, DMA halved, DVE 2-4x modes). Conv w1 is folded into
W_k/W_v host-side (rope is linear) making each conv one
scalar_tensor_tensor per head; masking is multiplicative post-exp;
softmax denominators come from bf16 DVE tile-folds + one [1,512]
ones-matmul per (q-chunk, kv head); V is conv'd in [d, tok] layout then
PE-transposed (bf16) into [tok, d] tiles for the PV matmul.

Layouts (per core, on-chip):
  qpair[i] [128d, 2*S]   roped Q, the 2 q-heads sharing kv head i
  kct[i][j] [128d, 128]  roped+conv'd K tile (w1k folded into weights)
  vt[i][j] [128tok,128d] conv'd V tile for token block j (w1v in weights)
  scoresT[k,q] = sum_d kT[d,k] qT[d,q]; outT[d,q] = sum_k v[k,d] pT[k,q]
"""

import numpy as np
import ml_dtypes

B, S, H = 2, 2048, 2048
NH, NKV, HD = 16, 8, 128
WINDOW = 1024
THETA = 100000.0
TP = 4                      # tensor-parallel ways (head groups)
QH = NH // TP               # 4 q heads per core
KVH = NKV // TP             # 2 kv heads per core
NCORES = 8
SCALE = 1.0 / float(np.sqrt(HD))

NT = S // 256               # 8 attention q-chunks of 256
NSC = S // 512              # 4 super-chunks of 512
NK = H // 128               # 16 contraction tiles

_CACHE = {}


def _build_program():
    import concourse.bacc as bacc
    import concourse.mybir as mybir
    import concourse.tile as tile

    f32 = mybir.dt.float32
    bf16 = mybir.dt.bfloat16
    Exp = mybir.ActivationFunctionType.Exp
    mult = mybir.AluOpType.mult
    add = mybir.AluOpType.add

    nc = bacc.Bacc("TRN2", target_bir_lowering=False, debug=False,
                   enable_asserts=False, num_devices=NCORES)

    hT_d = nc.dram_tensor("hT", [H, S], bf16, kind="ExternalInput")
    wpk_d = nc.dram_tensor("wpk", [H, 1024], bf16, kind="ExternalInput")
    wo_d = nc.dram_tensor("wo", [QH * HD, H], bf16, kind="ExternalInput")
    # per super-chunk: [cos dup x2 head-cols (1024) | +-sin dup (1024)]
    csn_d = nc.dram_tensor("csn", [128, NSC * 2048], bf16,
                           kind="ExternalInput")
    cw_d = nc.dram_tensor("cw", [128, 4], f32, kind="ExternalInput")
    # multiplicative bf16 masks: [m1024h 256 | m896 512 | m0 512 | mm128h 256]
    msk_d = nc.dram_tensor("msk", [128, 1536], bf16, kind="ExternalInput")
    one_d = nc.dram_tensor("one", [128, 1], bf16, kind="ExternalInput")
    eye_d = nc.dram_tensor("eye", [128, 128], bf16, kind="ExternalInput")
    yT_d = nc.dram_tensor("yT", [H, S], bf16, kind="ExternalOutput")

    with tile.TileContext(nc) as tc:
        with (
            tc.tile_pool(name="const", bufs=1) as cp,
            tc.tile_pool(name="wts", bufs=1) as wp,
            tc.tile_pool(name="persist", bufs=1) as pp,
            tc.tile_pool(name="ht", bufs=2) as bht,
            tc.tile_pool(name="roll", bufs=2) as rl,
            tc.tile_pool(name="ebuf", bufs=2) as eb,
            tc.tile_pool(name="pb", bufs=8) as pbp,
            tc.tile_pool(name="accp", bufs=4) as accp,
            tc.tile_pool(name="accf", bufs=3) as accfp,
            tc.tile_pool(name="rbp", bufs=3) as rbp,
            tc.tile_pool(name="ybp", bufs=3) as ybp,
            tc.tile_pool(name="mm", bufs=4, space="PSUM") as mm,
            tc.tile_pool(name="scp", bufs=2, space="PSUM") as scp,
            tc.tile_pool(name="pvp", bufs=2, space="PSUM") as pvp,
        ):
            # --- weight + first super-chunk loads, interleaved so the
            # DMA-bound startup overlaps: wf pairs alternate ht0 quarters
            wfc = [wp.tile([128, 2048], bf16, tag=f"wfc{p}", name=f"wfc{p}")
                   for p in range(NK // 2)]

            def wfs(k, lo, hi):
                o = (k % 2) * 1024
                return wfc[k // 2][:, o + lo:o + hi]

            htile = bht.tile([128, NK * 512], bf16, tag="htc", name="htc")
            htq_after = {0: 0, 1: 2, 2: 4, 3: 6}
            for p in range(NK // 2):
                nc.sync.dma_start(
                    out=wfc[p][:].rearrange("q (k c) -> q k c", k=2),
                    in_=wpk_d[p * 256:(p + 1) * 256, :].rearrange(
                        "(k q) c -> q k c", k=2))
                for q, after in htq_after.items():
                    if after == p:
                        nc.sync.dma_start(
                            out=htile[:, q * 2048:(q + 1) * 2048].rearrange(
                                "w (k s) -> w k s", k=4),
                            in_=hT_d[q * 512:(q + 1) * 512, 0:512].rearrange(
                                "(k w) s -> w k s", k=4))

            def load_cssn(sc):
                csn = rl.tile([128, 2048], bf16, tag="csr", name="csr")
                nc.sync.dma_start(out=csn[:],
                                  in_=csn_d[:, sc * 2048:(sc + 1) * 2048])
                return csn

            cssn = load_cssn(0)
            htile1 = bht.tile([128, NK * 512], bf16, tag="htc",
                              name="htc")
            nc.sync.dma_start(
                out=htile1[:].rearrange("p (k s) -> p k s", k=NK),
                in_=hT_d[:, 512:1024].rearrange("(k p) s -> p k s", k=NK))
            cw_sb = cp.tile([128, 4], f32, tag="cw", name="cw")
            msk_sb = cp.tile([128, 1536], bf16, tag="msk", name="msk")
            one_sb = cp.tile([128, 1], bf16, tag="one", name="one")
            eye_sb = cp.tile([128, 128], bf16, tag="eye", name="eye")
            nc.sync.dma_start(out=cw_sb[:], in_=cw_d[:, :])
            nc.sync.dma_start(out=msk_sb[:], in_=msk_d[:, :])
            nc.sync.dma_start(out=one_sb[:], in_=one_d[:, :])
            nc.sync.dma_start(out=eye_sb[:], in_=eye_d[:, :])
            wo_sb = wp.tile([128, QH * H], bf16, tag="wo", name="wo")

            # --- persistent activations (bf16)
            qpair = [pp.tile([128, 2 * S], bf16, tag=f"qp{i}", name=f"qp{i}")
                     for i in range(KVH)]
            kct = [[pp.tile([128, 128], bf16, tag=f"kc{i}_{j}",
                            name=f"kc{i}_{j}") for j in range(NK)]
                   for i in range(KVH)]
            vt = [[pp.tile([128, 128], bf16, tag=f"vt{i}_{j}",
                           name=f"vt{i}_{j}") for j in range(NK)]
                  for i in range(KVH)]
            attn = [pp.tile([128, S], bf16, tag=f"at{h}", name=f"at{h}")
                    for h in range(QH)]

            M1024H, M896, M0, MM128H = 0, 256, 768, 1280

            def make_sum_step(st, qc, i):
                """Lazy: reads st['acc']/st['ps_o'] at emission time."""
                def emit():
                    ps_s = scp.tile([128, 512], f32, tag="sc", name="sc")
                    nc.tensor.matmul(ps_s[0:1, :], one_sb[:], st["acc"][:],
                                     start=True, stop=True)
                    rsum = eb.tile([1, 512], f32, tag="rs", name="rs")
                    nc.vector.reciprocal(rsum[:], ps_s[0:1, :])
                    rb = rbp.tile([128, 512], f32, tag="rb", name="rb")
                    nc.gpsimd.partition_broadcast(rb[:], rsum[:])
                    nc.vector.tensor_mul(attn[2 * i][:, qc:qc + 256],
                                         st["ps_o"][:, 0:256], rb[:, 0:256])
                    nc.vector.tensor_mul(attn[2 * i + 1][:, qc:qc + 256],
                                         st["ps_o"][:, 256:512],
                                         rb[:, 256:512])
                return emit

            def attention_steps(qi, carry_in, last=False):
                """Step closures for q-chunk qi. carry_in: the previous
                head's softmax-denominator step, woven in near the start of
                head 0's stream. Returns (steps, carry_out)."""
                steps = []
                carry = carry_in
                qc = qi * 256
                for i in range(KVH):
                    st = {"acc": None, "ps_o": None, "po3": None}
                    q2 = qpair[i][:].rearrange("p (h s) -> p h s", h=2)
                    jstart = max(0, qc // 128 - 8)
                    jend = qc // 128 + 1
                    fulls = [j for j in range(jstart, jend)
                             if qc - j * 128 != 1024]
                    halves = ([j for j in range(jstart, jend)
                               if qc - j * 128 == 1024] + [jend])
                    tiles = ([("f", j) for j in fulls] +
                             [("h", j) for j in halves])
                    ntile = len(tiles)

                    def a_step(kind, j, idx, st=st, i=i, qc=qc, q2=q2,
                               ntile=ntile):
                        delta = qc - j * 128
                        ps_sc = scp.tile([128, 512], f32, tag="sc", name="sc")
                        if kind == "f":
                            nc.tensor.matmul(
                                ps_sc[:], kct[i][j][:],
                                q2[:, :, qc:qc + 256],
                                start=True, stop=True)
                            pb = pbp.tile([128, 512], bf16, tag="pb",
                                          name="pb")
                            if idx == 0 and delta not in (896, 0):
                                acc = accp.tile([128, 512], bf16, tag="acc",
                                                name="acc")
                                nc.scalar.activation(acc[:], ps_sc[:], Exp,
                                                     bias=0.0, scale=SCALE)
                                st["acc"] = acc
                                st["pb"] = acc
                                return
                            nc.scalar.activation(pb[:], ps_sc[:], Exp,
                                                 bias=0.0, scale=SCALE)
                            if delta in (896, 0):
                                moff = M896 if delta == 896 else M0
                                if idx == 0:
                                    acc = accp.tile([128, 512], bf16,
                                                    tag="acc", name="acc")
                                    nc.vector.tensor_mul(
                                        acc[:], pb[:],
                                        msk_sb[:, moff:moff + 512])
                                    st["acc"] = acc
                                    st["pb"] = acc
                                    return
                                nc.vector.tensor_mul(
                                    pb[:], pb[:],
                                    msk_sb[:, moff:moff + 512])
                            nacc = accp.tile([128, 512], bf16, tag="acc",
                                             name="acc")
                            nc.vector.tensor_add(nacc[:], pb[:],
                                                 st["acc"][:])
                            st["acc"] = nacc
                            st["pb"] = pb
                        else:
                            qoff = 0 if delta == 1024 else 128
                            moff = M1024H if delta == 1024 else MM128H
                            ps3 = ps_sc[:].rearrange("p (h q) -> p h q", h=2)
                            nc.tensor.matmul(
                                ps3[:, :, 0:128], kct[i][j][:],
                                q2[:, :, qc + qoff:qc + qoff + 128],
                                start=True, stop=True)
                            pb = pbp.tile([128, 512], bf16, tag="pb",
                                          name="pb")
                            pb3 = pb[:].rearrange("p (h q) -> p h q", h=2)
                            nc.scalar.activation(pb3[:, :, 0:128],
                                                 ps3[:, :, 0:128], Exp,
                                                 bias=0.0, scale=SCALE)
                            m3 = msk_sb[:, moff:moff + 256].rearrange(
                                "p (h q) -> p h q", h=2)
                            nc.vector.tensor_mul(pb3[:, :, 0:128],
                                                 pb3[:, :, 0:128],
                                                 m3[:, :, :])
                            last_fold = (idx == ntile - 1)
                            pool = accfp if last_fold else accp
                            tag = "accf" if last_fold else "acc"
                            nacc = pool.tile([128, 512], bf16, tag=tag,
                                             name=tag)
                            acc3 = st["acc"][:].rearrange(
                                "p (h q) -> p h q", h=2)
                            nacc3 = nacc[:].rearrange("p (h q) -> p h q", h=2)
                            nc.vector.tensor_add(
                                nacc3[:, :, qoff:qoff + 128],
                                pb3[:, :, 0:128],
                                acc3[:, :, qoff:qoff + 128])
                            oq = 128 - qoff
                            nc.vector.tensor_copy(
                                nacc3[:, :, oq:oq + 128],
                                acc3[:, :, oq:oq + 128])
                            st["acc"] = nacc
                            st["pb"] = pb

                    def b_step(kind, j, idx, st=st, i=i, ntile=ntile, qc=qc):
                        first = (idx == 0)
                        fin = (idx == ntile - 1)
                        if first:
                            st["ps_o"] = pvp.tile([128, 512], f32, tag="pv",
                                                  name="pv")
                            st["po3"] = st["ps_o"][:].rearrange(
                                "p (h q) -> p h q", h=2)
                        pb = st[f"pb{idx}"]
                        if kind == "f":
                            nc.tensor.matmul(st["ps_o"][:], vt[i][j][:],
                                             pb[:], start=first, stop=fin)
                        else:
                            delta = qc - j * 128
                            qoff = 0 if delta == 1024 else 128
                            pb3 = pb[:].rearrange("p (h q) -> p h q", h=2)
                            nc.tensor.matmul(
                                st["po3"][:, :, qoff:qoff + 128],
                                vt[i][j][:], pb3[:, :, 0:128],
                                start=False, stop=fin)

                    def make_a(kind, j, idx, a_step=a_step, st=st):
                        def f():
                            a_step(kind, j, idx)
                            st[f"pb{idx}"] = st["pb"]
                        return f

                    def make_b(kind, j, idx, b_step=b_step):
                        return lambda: b_step(kind, j, idx)

                    seq = []
                    seq.append(make_a(*tiles[0], 0))
                    if ntile > 1:
                        seq.append(make_a(*tiles[1], 1))
                    if carry is not None:
                        seq.append(carry)
                        carry = None
                    seq.append(make_b(*tiles[0], 0))
                    for idx in range(2, ntile):
                        seq.append(make_a(*tiles[idx], idx))
                        seq.append(make_b(*tiles[idx - 1], idx - 1))
                    if ntile > 1:
                        seq.append(make_b(*tiles[ntile - 1], ntile - 1))
                    sum_step = make_sum_step(st, qc, i)
                    if last and i == KVH - 1:
                        seq.append(sum_step)
                        carry = None
                    else:
                        carry = sum_step
                    steps.extend(seq)
                return steps, carry

            def oproj_units(sc, qhalf=None, alt=False):
                """16 units: 4 accumulating matmuls over [128,512] tokens
                (one full psum bank) + psum->sbuf copy + one DMA out.
                qhalf selects a 256-token half (for the tail)."""
                lo = sc * 512 + (0 if qhalf in (None, 0) else 256)
                w = 512 if qhalf is None else 256
                units = []
                for oc in range(NK):
                    def unit(oc=oc, sc=sc, lo=lo, w=w, alt=alt):
                        ps_y = mm.tile([128, 512], f32, tag="qkv", name="op")
                        for d in range(QH):
                            nc.tensor.matmul(
                                ps_y[:, 0:w],
                                wo_sb[:, d * H + oc * 128:
                                      d * H + (oc + 1) * 128],
                                attn[d][:, lo:lo + w],
                                start=(d == 0), stop=(d == QH - 1))
                        yb = ybp.tile([128, 512], bf16, tag="yb",
                                      name="yb")
                        if alt and oc % 2 == 1:
                            nc.vector.tensor_copy(yb[:, 0:w], ps_y[:, 0:w])
                        else:
                            nc.scalar.copy(yb[:, 0:w], ps_y[:, 0:w])
                        nc.sync.dma_start(
                            out=yT_d[oc * 128:(oc + 1) * 128, lo:lo + w],
                            in_=yb[:, 0:w])
                    units.append(unit)
                return units

            krprev = [None]
            vrawprev = [None]

            def rope_pair(sc, cp_, srcs, cssn_t, kr):
                """srcs: two [128,512] col tensors. cp_<2 -> qpair[cp_];
                cp_==2 -> kr. out = src*cos + swap64(src)*(+-sin)."""
                csl = cssn_t[:, 0:1024]
                snl = cssn_t[:, 1024:2048]
                if cp_ < 2:
                    dst3 = qpair[cp_][:].rearrange(
                        "p (h s) -> p h s", h=2)[:, :,
                                                 sc * 512:(sc + 1) * 512]
                else:
                    dst3 = kr[:].rearrange("p (h s) -> p h s", h=2)
                e1 = eb.tile([128, 1024], f32, tag="e1", name="e1")
                e2 = eb.tile([128, 1024], f32, tag="e2", name="e2")
                for h in range(2):
                    sl = slice(h * 512, (h + 1) * 512)
                    nc.vector.tensor_mul(e1[:, sl], srcs[h][:], csl[:, sl])
                    nc.vector.tensor_mul(e2[0:64, sl], srcs[h][64:128, :],
                                         snl[0:64, sl])
                    nc.vector.tensor_mul(e2[64:128, sl], srcs[h][0:64, :],
                                         snl[64:128, sl])
                e13 = e1[:].rearrange("p (h s) -> p h s", h=2)
                e23 = e2[:].rearrange("p (h s) -> p h s", h=2)
                nc.gpsimd.tensor_add(dst3[:, :, :], e13[:, :, :],
                                     e23[:, :, :])

            def kconv_emit(sc, kr):
                # kconv = kr + (w0k/w1k) * kr_prev-token (w1k in weights)
                for i in range(KVH):
                    r = cw_sb[:, i:i + 1]
                    b = i * 512
                    for jj in range(4):
                        j = 4 * sc + jj
                        lo = b + jj * 128
                        if jj == 0:
                            nc.vector.scalar_tensor_tensor(
                                kct[i][j][:, 1:128], kr[:, b:b + 127], r,
                                kr[:, b + 1:b + 128], mult, add)
                            if sc == 0:
                                nc.vector.tensor_copy(kct[i][j][:, 0:1],
                                                      kr[:, b:b + 1])
                            else:
                                nc.vector.scalar_tensor_tensor(
                                    kct[i][j][:, 0:1],
                                    krprev[0][:, b + 511:b + 512], r,
                                    kr[:, b:b + 1], mult, add)
                        else:
                            nc.vector.scalar_tensor_tensor(
                                kct[i][j][:, 0:128],
                                kr[:, lo - 1:lo + 127], r,
                                kr[:, lo:lo + 128], mult, add)
                krprev[0] = kr

            def vconv_pre(sc, psV):
                """psV: 2 psum tiles [128 d, 512 tok] (one per kv head).
                Copy to SBUF bf16, conv along tokens. Returns vcb."""
                vcb = rl.tile([128, 1024], bf16, tag="vcb", name="vcb")
                vraw = rl.tile([128, 1024], bf16, tag="vraw", name="vraw")
                for i in range(KVH):
                    b = i * 512
                    nc.scalar.copy(vraw[:, b:b + 512], psV[i][:])
                    r = cw_sb[:, 2 + i:3 + i]
                    nc.vector.scalar_tensor_tensor(
                        vcb[:, b + 1:b + 512], vraw[:, b:b + 511], r,
                        vraw[:, b + 1:b + 512], mult, add)
                    if sc == 0:
                        nc.vector.tensor_copy(vcb[:, b:b + 1],
                                              vraw[:, b:b + 1])
                    else:
                        nc.vector.scalar_tensor_tensor(
                            vcb[:, b:b + 1],
                            vrawprev[0][:, b + 511:b + 512], r,
                            vraw[:, b:b + 1], mult, add)
                vrawprev[0] = vraw
                return vcb

            def vconv_transposes(sc, vcb):
                for half in range(2):
                    tps = scp.tile([128, 512], f32, tag="sc", name="vtp")
                    tpb = tps[:].bitcast(bf16)
                    for u in range(4):
                        g = 4 * half + u      # global 128-block 0..7
                        i, sub = g // 4, g % 4
                        nc.tensor.transpose(
                            tpb[:, u * 128:(u + 1) * 128],
                            vcb[:, i * 512 + sub * 128:
                                i * 512 + (sub + 1) * 128],
                            eye_sb[:])
                        nc.vector.tensor_copy(vt[i][4 * sc + sub][:],
                                              tpb[:, u * 128:(u + 1) * 128])

            carry = None

            # ---------------- main software-pipelined loop ----------------
            for t in range(NSC + 1):
                if t >= 1:
                    sa, carry = attention_steps(2 * (t - 1), carry)
                    sb, carry = attention_steps(
                        2 * (t - 1) + 1, carry,
                        last=(2 * (t - 1) + 1 == NT - 1))
                    steps = sa + sb
                else:
                    steps = []
                units = (oproj_units(t - 2, alt=(t == NSC))
                         if t >= 2 else [])
                if t < NSC:
                    fill = steps
                else:
                    fill = steps[:4]
                    rest = steps[4:]
                    for u in range(max(len(units), len(rest))):
                        if u < len(units):
                            fill.append(units[u])
                        if u < len(rest):
                            fill.append(rest[u])
                si = 0

                if t < NSC:
                    if t + 1 < NSC:
                        if t == 0:
                            nhtile = htile1
                        else:
                            nhtile = bht.tile([128, NK * 512], bf16,
                                              tag="htc", name="htc")
                            nc.sync.dma_start(
                                out=nhtile[:].rearrange("p (k s) -> p k s",
                                                        k=NK),
                                in_=hT_d[:, (t + 1) * 512:(t + 2) * 512]
                                .rearrange("(k p) s -> p k s", k=NK))
                        ncssn = load_cssn(t + 1)
                    if t == 0:
                        nc.sync.dma_start(
                            out=wo_sb[:].rearrange("p (d c) -> p d c", d=QH),
                            in_=wo_d[:, :].rearrange("(d p) c -> p d c",
                                                     d=QH))

                    # --- qkv: col-outer, k-inner; one group per full bank.
                    # cols 0..3 = Q heads, 4..5 = K heads, 6..7 = V heads.
                    col_src = [None] * 8
                    nslot = 8 * NK
                    slot = 0
                    kr = rl.tile([128, 1024], bf16, tag="kr", name="kr")
                    vcb = None
                    if True:
                        for col in range(8):
                            ps = mm.tile([128, 512], f32, tag="qkv",
                                         name=f"ps{col}")
                            for k in range(NK):
                                nc.tensor.matmul(
                                    ps[:],
                                    wfs(k, col * 128, (col + 1) * 128),
                                    htile[:, k * 512:(k + 1) * 512],
                                    start=(k == 0), stop=(k == NK - 1))
                                slot += 1
                                want = slot * len(fill) * 3 // (4 * nslot)
                                while si < min(want, len(fill)):
                                    fill[si]()
                                    si += 1
                            col_src[col] = ps
                            if col == 1:
                                rope_pair(t, 0, col_src[0:2], cssn, None)
                            elif col == 3:
                                rope_pair(t, 1, col_src[2:4], cssn, None)
                            elif col == 5:
                                rope_pair(t, 2, col_src[4:6], cssn, kr)
                                kconv_emit(t, kr)
                            elif col == 7:
                                vcb = vconv_pre(t, col_src[6:8])
                    nu = len(units)
                    rest = len(fill) - si
                    for ui in range(max(nu, 1)):
                        if ui < nu:
                            units[ui]()
                        tgt = si + (rest // max(nu, 1)) if ui < nu - 1 \
                            else len(fill)
                        while si < min(tgt, len(fill)):
                            fill[si]()
                            si += 1
                    vconv_transposes(t, vcb)
                    if t + 1 < NSC:
                        htile = nhtile
                        cssn = ncssn
                else:
                    while si < len(fill):
                        fill[si]()
                        si += 1
                    for u in oproj_units(NSC - 1, alt=True):
                        u()

    nc.finalize()
    return nc


def _host_inputs(hidden, W_pack, W_o, conv_k, conv_v):
    """Per-core input maps."""
    bf16 = ml_dtypes.bfloat16
    pos = np.arange(S, dtype=np.float64)
    inv_freq = 1.0 / (THETA ** (np.arange(0, HD, 2, dtype=np.float64) / HD))
    freqs = np.outer(pos, inv_freq)                       # (S, 64)
    cos = np.cos(freqs).T.astype(np.float32)              # (64, S)
    sin = np.sin(freqs).T.astype(np.float32)
    cos = np.concatenate([cos, cos], axis=0)              # (128, S)
    sin = np.concatenate([-sin, sin], axis=0)             # sign folded in
    # per super-chunk: [cos dup x2 head-cols (1024) | +-sin dup (1024)]
    cs = np.broadcast_to(
        cos.reshape(128, NSC, 1, 512),
        (128, NSC, 2, 512)).reshape(128, NSC, 1024)
    sn = np.broadcast_to(
        sin.reshape(128, NSC, 1, 512),
        (128, NSC, 2, 512)).reshape(128, NSC, 1024)
    csn = np.concatenate([cs, sn], axis=2).reshape(128, -1)
    csn = np.ascontiguousarray(csn).astype(bf16)

    kk = np.arange(128)[:, None]
    qq = np.arange(128)[None, :]
    qq2 = np.arange(256)[None, :]

    def double(m):
        return np.concatenate([m, m], axis=1)
    m1024h = double(qq < kk)                       # [128, 256]
    m896 = double(qq2 - kk < 128)                  # [128, 512]
    m0 = double(qq2 >= kk)                         # [128, 512]
    mm128h = double(qq >= kk)                      # [128, 256]
    msk = np.concatenate([m1024h, m896, m0, mm128h],
                         axis=1).astype(bf16)      # [128, 1536]

    one = np.ones((128, 1), bf16)
    eye = np.eye(128).astype(bf16)

    in_maps = []
    for c in range(NCORES):
        b, g = c // TP, c % TP
        hT = np.ascontiguousarray(hidden[b].T).astype(bf16)
        wq = W_pack[:, g * 512:(g + 1) * 512]
        wk = W_pack[:, NH * HD + 2 * g * 128: NH * HD + (2 * g + 2) * 128]
        wv = W_pack[:, NH * HD + NKV * HD + 2 * g * 128:
                    NH * HD + NKV * HD + (2 * g + 2) * 128]
        # fold conv w1 into Wk/Wv (rope is linear; conv comes after rope)
        wk = wk.copy()
        wv = wv.copy()
        for i in range(KVH):
            wk[:, i * 128:(i + 1) * 128] *= conv_k[2 * g + i, 1]
            wv[:, i * 128:(i + 1) * 128] *= conv_v[2 * g + i, 1]
        wpk = np.ascontiguousarray(
            np.concatenate([wq, wk, wv], axis=1)).astype(bf16)
        wo = np.ascontiguousarray(
            W_o[g * 512:(g + 1) * 512, :]).astype(bf16)
        cwv = np.empty(4, np.float32)
        for i in range(KVH):
            cwv[i] = conv_k[2 * g + i, 0] / conv_k[2 * g + i, 1]
            cwv[2 + i] = conv_v[2 * g + i, 0] / conv_v[2 * g + i, 1]
        cw = np.broadcast_to(cwv, (128, 4)).copy()
        in_maps.append({
            "hT": hT, "wpk": wpk, "wo": wo, "csn": csn,
            "cw": cw, "msk": msk, "one": one, "eye": eye,
        })
    return in_maps


def run_cores(in_maps, trace=False, **kw):
    from concourse.bass_utils import run_bass_kernel_spmd
    if "nc" not in _CACHE:
        _CACHE["nc"] = _build_program()
    return run_bass_kernel_spmd(_CACHE["nc"], in_maps, list(range(NCORES)),
                                trace=trace, **kw)


def kernel(hidden, W_pack, W_o, conv_k, conv_v):
    hidden = np.asarray(hidden, np.float32)
    W_pack = np.asarray(W_pack, np.float32)
    W_o = np.asarray(W_o, np.float32)
    conv_k = np.asarray(conv_k, np.float32)
    conv_v = np.asarray(conv_v, np.float32)
    in_maps = _host_inputs(hidden, W_pack, W_o, conv_k, conv_v)
    res = run_cores(in_maps)
    out = np.zeros((B, S, H), np.float32)
    for c in range(NCORES):
        b = c // TP
        out[b] += res.results[c]["yT"].T.astype(np.float32)
    return out


# revision 57
# speedup vs baseline: 1.0428x; 1.0421x over previous
"""Baichuan sliding-window GQA attention block on 8 trn2 NeuronCores.

Sharding: data-parallel over batch (2) x tensor-parallel over heads (4).
Core c handles batch b=c//4, head group g=c%4 (q heads 4g..4g+3, kv heads
2g..2g+1). Each core computes qkv projection, RoPE, 2-tap causal conv,
windowed attention and a row-sharded o_proj partial; the host sums the 4
partials per batch.

v3: software-pipelined 512-token super-chunk loop. Iteration t emits,
interleaved at instruction level: qkv matmuls for super-chunk t,
attention tile-steps for super-chunk t-1 (two 256-token q-chunks), and
o_proj for super-chunk t-2 — so the PE always has independent GEMM work
between an attention tile's scores (PE) -> exp (ACT) -> mask/fold (DVE)
-> pv (PE) chain. Every PSUM accumulation group covers exactly one full
2KB bank (a matmul group start zeroes its entire bank on TRN2, so banks
cannot be shared between concurrent groups). bf16 storage everywhere
(PE 1 cyc/row, DMA halved, DVE 2-4x modes). Conv w1 is folded into
W_k/W_v host-side (rope is linear) making each conv one
scalar_tensor_tensor per head; masking is multiplicative post-exp;
softmax denominators come from bf16 DVE tile-folds + one [1,512]
ones-matmul per (q-chunk, kv head); V is conv'd in [d, tok] layout then
PE-transposed (bf16) into [tok, d] tiles for the PV matmul.

Layouts (per core, on-chip):
  qpair[i] [128d, 2*S]   roped Q, the 2 q-heads sharing kv head i
  kct[i][j] [128d, 128]  roped+conv'd K tile (w1k folded into weights)
  vt[i][j] [128tok,128d] conv'd V tile for token block j (w1v in weights)
  scoresT[k,q] = sum_d kT[d,k] qT[d,q]; outT[d,q] = sum_k v[k,d] pT[k,q]
"""

import numpy as np
import ml_dtypes

B, S, H = 2, 2048, 2048
NH, NKV, HD = 16, 8, 128
WINDOW = 1024
THETA = 100000.0
TP = 4                      # tensor-parallel ways (head groups)
QH = NH // TP               # 4 q heads per core
KVH = NKV // TP             # 2 kv heads per core
NCORES = 8
SCALE = 1.0 / float(np.sqrt(HD))

NT = S // 256               # 8 attention q-chunks of 256
NSC = S // 512              # 4 super-chunks of 512
NK = H // 128               # 16 contraction tiles

_CACHE = {}


def _build_program():
    import concourse.bacc as bacc
    import concourse.mybir as mybir
    import concourse.tile as tile

    f32 = mybir.dt.float32
    bf16 = mybir.dt.bfloat16
    Exp = mybir.ActivationFunctionType.Exp
    mult = mybir.AluOpType.mult
    add = mybir.AluOpType.add

    nc = bacc.Bacc("TRN2", target_bir_lowering=False, debug=False,
                   enable_asserts=False, num_devices=NCORES)

    hT_d = nc.dram_tensor("hT", [H, S], bf16, kind="ExternalInput")
    wpk_d = nc.dram_tensor("wpk", [H, 1024], bf16, kind="ExternalInput")
    wo_d = nc.dram_tensor("wo", [QH * HD, H], bf16, kind="ExternalInput")
    # per super-chunk: [cos dup x2 head-cols (1024) | +-sin dup (1024)]
    csn_d = nc.dram_tensor("csn", [128, NSC * 2048], bf16,
                           kind="ExternalInput")
    cw_d = nc.dram_tensor("cw", [128, 4], f32, kind="ExternalInput")
    # multiplicative bf16 masks: [m1024h 256 | m896 512 | m0 512 | mm128h 256]
    msk_d = nc.dram_tensor("msk", [128, 1536], bf16, kind="ExternalInput")
    one_d = nc.dram_tensor("one", [128, 1], bf16, kind="ExternalInput")
    eye_d = nc.dram_tensor("eye", [128, 128], bf16, kind="ExternalInput")
    yT_d = nc.dram_tensor("yT", [H, S], bf16, kind="ExternalOutput")

    with tile.TileContext(nc) as tc:
        with (
            tc.tile_pool(name="const", bufs=1) as cp,
            tc.tile_pool(name="wts", bufs=1) as wp,
            tc.tile_pool(name="persist", bufs=1) as pp,
            tc.tile_pool(name="ht", bufs=2) as bht,
            tc.tile_pool(name="roll", bufs=2) as rl,
            tc.tile_pool(name="ebuf", bufs=3) as eb,
            tc.tile_pool(name="pb", bufs=8) as pbp,
            tc.tile_pool(name="accp", bufs=4) as accp,
            tc.tile_pool(name="accf", bufs=3) as accfp,
            tc.tile_pool(name="rbp", bufs=3) as rbp,
            tc.tile_pool(name="ybp", bufs=4) as ybp,
            tc.tile_pool(name="mm", bufs=4, space="PSUM") as mm,
            tc.tile_pool(name="scp", bufs=2, space="PSUM") as scp,
            tc.tile_pool(name="pvp", bufs=2, space="PSUM") as pvp,
        ):
            # --- weight + first super-chunk loads, interleaved so the
            # DMA-bound startup overlaps: wf pairs alternate ht0 quarters
            wfc = [wp.tile([128, 2048], bf16, tag=f"wfc{p}", name=f"wfc{p}")
                   for p in range(NK // 2)]

            def wfs(k, lo, hi):
                o = (k % 2) * 1024
                return wfc[k // 2][:, o + lo:o + hi]

            htile = bht.tile([128, NK * 512], bf16, tag="htc", name="htc")
            htq_after = {0: 0, 1: 2, 2: 4, 3: 6}
            for p in range(NK // 2):
                nc.sync.dma_start(
                    out=wfc[p][:].rearrange("q (k c) -> q k c", k=2),
                    in_=wpk_d[p * 256:(p + 1) * 256, :].rearrange(
                        "(k q) c -> q k c", k=2))
                for q, after in htq_after.items():
                    if after == p:
                        nc.sync.dma_start(
                            out=htile[:, q * 2048:(q + 1) * 2048].rearrange(
                                "w (k s) -> w k s", k=4),
                            in_=hT_d[q * 512:(q + 1) * 512, 0:512].rearrange(
                                "(k w) s -> w k s", k=4))

            def load_cssn(sc):
                csn = rl.tile([128, 2048], bf16, tag="csr", name="csr")
                nc.sync.dma_start(out=csn[:],
                                  in_=csn_d[:, sc * 2048:(sc + 1) * 2048])
                return csn

            cssn = load_cssn(0)
            htile1 = bht.tile([128, NK * 512], bf16, tag="htc",
                              name="htc")
            nc.sync.dma_start(
                out=htile1[:].rearrange("p (k s) -> p k s", k=NK),
                in_=hT_d[:, 512:1024].rearrange("(k p) s -> p k s", k=NK))
            cw_sb = cp.tile([128, 4], f32, tag="cw", name="cw")
            msk_sb = cp.tile([128, 1536], bf16, tag="msk", name="msk")
            one_sb = cp.tile([128, 1], bf16, tag="one", name="one")
            eye_sb = cp.tile([128, 128], bf16, tag="eye", name="eye")
            nc.sync.dma_start(out=cw_sb[:], in_=cw_d[:, :])
            nc.sync.dma_start(out=msk_sb[:], in_=msk_d[:, :])
            nc.sync.dma_start(out=one_sb[:], in_=one_d[:, :])
            nc.sync.dma_start(out=eye_sb[:], in_=eye_d[:, :])
            wo_sb = wp.tile([128, QH * H], bf16, tag="wo", name="wo")

            # --- persistent activations (bf16)
            qpair = [pp.tile([128, 2 * S], bf16, tag=f"qp{i}", name=f"qp{i}")
                     for i in range(KVH)]
            kct = [[pp.tile([128, 128], bf16, tag=f"kc{i}_{j}",
                            name=f"kc{i}_{j}") for j in range(NK)]
                   for i in range(KVH)]
            vt = [[pp.tile([128, 128], bf16, tag=f"vt{i}_{j}",
                           name=f"vt{i}_{j}") for j in range(NK)]
                  for i in range(KVH)]
            attn = [pp.tile([128, S], bf16, tag=f"at{h}", name=f"at{h}")
                    for h in range(QH)]

            M1024H, M896, M0, MM128H = 0, 256, 768, 1280

            def make_sum_step(st, qc, i):
                """Lazy: reads st['acc']/st['ps_o'] at emission time."""
                def emit():
                    ps_s = scp.tile([128, 512], f32, tag="sc", name="sc")
                    nc.tensor.matmul(ps_s[0:1, :], one_sb[:], st["acc"][:],
                                     start=True, stop=True)
                    rsum = eb.tile([1, 512], f32, tag="rs", name="rs")
                    nc.vector.reciprocal(rsum[:], ps_s[0:1, :])
                    rb = rbp.tile([128, 512], f32, tag="rb", name="rb")
                    nc.gpsimd.partition_broadcast(rb[:], rsum[:])
                    nc.vector.tensor_mul(attn[2 * i][:, qc:qc + 256],
                                         st["ps_o"][:, 0:256], rb[:, 0:256])
                    nc.vector.tensor_mul(attn[2 * i + 1][:, qc:qc + 256],
                                         st["ps_o"][:, 256:512],
                                         rb[:, 256:512])
                return emit

            def attention_steps(qi, carry_in, last=False):
                """Step closures for q-chunk qi. carry_in: the previous
                head's softmax-denominator step, woven in near the start of
                head 0's stream. Returns (steps, carry_out)."""
                steps = []
                carry = carry_in
                qc = qi * 256
                for i in range(KVH):
                    st = {"acc": None, "ps_o": None, "po3": None}
                    q2 = qpair[i][:].rearrange("p (h s) -> p h s", h=2)
                    jstart = max(0, qc // 128 - 8)
                    jend = qc // 128 + 1
                    fulls = [j for j in range(jstart, jend)
                             if qc - j * 128 != 1024]
                    halves = ([j for j in range(jstart, jend)
                               if qc - j * 128 == 1024] + [jend])
                    tiles = ([("f", j) for j in fulls] +
                             [("h", j) for j in halves])
                    ntile = len(tiles)

                    def a_step(kind, j, idx, st=st, i=i, qc=qc, q2=q2,
                               ntile=ntile):
                        delta = qc - j * 128
                        ps_sc = scp.tile([128, 512], f32, tag="sc", name="sc")
                        if kind == "f":
                            nc.tensor.matmul(
                                ps_sc[:], kct[i][j][:],
                                q2[:, :, qc:qc + 256],
                                start=True, stop=True)
                            pb = pbp.tile([128, 512], bf16, tag="pb",
                                          name="pb")
                            if idx == 0 and delta not in (896, 0):
                                acc = accp.tile([128, 512], bf16, tag="acc",
                                                name="acc")
                                nc.scalar.activation(acc[:], ps_sc[:], Exp,
                                                     bias=0.0, scale=SCALE)
                                st["acc"] = acc
                                st["pb"] = acc
                                return
                            nc.scalar.activation(pb[:], ps_sc[:], Exp,
                                                 bias=0.0, scale=SCALE)
                            if delta in (896, 0):
                                moff = M896 if delta == 896 else M0
                                if idx == 0:
                                    acc = accp.tile([128, 512], bf16,
                                                    tag="acc", name="acc")
                                    nc.vector.tensor_mul(
                                        acc[:], pb[:],
                                        msk_sb[:, moff:moff + 512])
                                    st["acc"] = acc
                                    st["pb"] = acc
                                    return
                                nc.vector.tensor_mul(
                                    pb[:], pb[:],
                                    msk_sb[:, moff:moff + 512])
                            nacc = accp.tile([128, 512], bf16, tag="acc",
                                             name="acc")
                            nc.vector.tensor_add(nacc[:], pb[:],
                                                 st["acc"][:])
                            st["acc"] = nacc
                            st["pb"] = pb
                        else:
                            qoff = 0 if delta == 1024 else 128
                            moff = M1024H if delta == 1024 else MM128H
                            ps3 = ps_sc[:].rearrange("p (h q) -> p h q", h=2)
                            nc.tensor.matmul(
                                ps3[:, :, 0:128], kct[i][j][:],
                                q2[:, :, qc + qoff:qc + qoff + 128],
                                start=True, stop=True)
                            pb = pbp.tile([128, 512], bf16, tag="pb",
                                          name="pb")
                            pb3 = pb[:].rearrange("p (h q) -> p h q", h=2)
                            nc.scalar.activation(pb3[:, :, 0:128],
                                                 ps3[:, :, 0:128], Exp,
                                                 bias=0.0, scale=SCALE)
                            m3 = msk_sb[:, moff:moff + 256].rearrange(
                                "p (h q) -> p h q", h=2)
                            nc.vector.tensor_mul(pb3[:, :, 0:128],
                                                 pb3[:, :, 0:128],
                                                 m3[:, :, :])
                            last_fold = (idx == ntile - 1)
                            pool = accfp if last_fold else accp
                            tag = "accf" if last_fold else "acc"
                            nacc = pool.tile([128, 512], bf16, tag=tag,
                                             name=tag)
                            acc3 = st["acc"][:].rearrange(
                                "p (h q) -> p h q", h=2)
                            nacc3 = nacc[:].rearrange("p (h q) -> p h q", h=2)
                            nc.vector.tensor_add(
                                nacc3[:, :, qoff:qoff + 128],
                                pb3[:, :, 0:128],
                                acc3[:, :, qoff:qoff + 128])
                            oq = 128 - qoff
                            nc.vector.tensor_copy(
                                nacc3[:, :, oq:oq + 128],
                                acc3[:, :, oq:oq + 128])
                            st["acc"] = nacc
                            st["pb"] = pb

                    def b_step(kind, j, idx, st=st, i=i, ntile=ntile, qc=qc):
                        first = (idx == 0)
                        fin = (idx == ntile - 1)
                        if first:
                            st["ps_o"] = pvp.tile([128, 512], f32, tag="pv",
                                                  name="pv")
                            st["po3"] = st["ps_o"][:].rearrange(
                                "p (h q) -> p h q", h=2)
                        pb = st[f"pb{idx}"]
                        if kind == "f":
                            nc.tensor.matmul(st["ps_o"][:], vt[i][j][:],
                                             pb[:], start=first, stop=fin)
                        else:
                            delta = qc - j * 128
                            qoff = 0 if delta == 1024 else 128
                            pb3 = pb[:].rearrange("p (h q) -> p h q", h=2)
                            nc.tensor.matmul(
                                st["po3"][:, :, qoff:qoff + 128],
                                vt[i][j][:], pb3[:, :, 0:128],
                                start=False, stop=fin)

                    def make_a(kind, j, idx, a_step=a_step, st=st):
                        def f():
                            a_step(kind, j, idx)
                            st[f"pb{idx}"] = st["pb"]
                        return f

                    def make_b(kind, j, idx, b_step=b_step):
                        return lambda: b_step(kind, j, idx)

                    seq = []
                    seq.append(make_a(*tiles[0], 0))
                    if ntile > 1:
                        seq.append(make_a(*tiles[1], 1))
                    if carry is not None:
                        seq.append(carry)
                        carry = None
                    seq.append(make_b(*tiles[0], 0))
                    for idx in range(2, ntile):
                        seq.append(make_a(*tiles[idx], idx))
                        seq.append(make_b(*tiles[idx - 1], idx - 1))
                    if ntile > 1:
                        seq.append(make_b(*tiles[ntile - 1], ntile - 1))
                    sum_step = make_sum_step(st, qc, i)
                    if last and i == KVH - 1:
                        seq.append(sum_step)
                        carry = None
                    else:
                        carry = sum_step
                    steps.extend(seq)
                return steps, carry

            def oproj_units(sc, qhalf=None, alt=False):
                """16 units: 4 accumulating matmuls over [128,512] tokens
                (one full psum bank) + psum->sbuf copy + one DMA out.
                qhalf selects a 256-token half (for the tail)."""
                lo = sc * 512 + (0 if qhalf in (None, 0) else 256)
                w = 512 if qhalf is None else 256
                units = []
                for oc in range(NK):
                    def unit(oc=oc, sc=sc, lo=lo, w=w, alt=alt):
                        ps_y = mm.tile([128, 512], f32, tag="qkv", name="op")
                        for d in range(QH):
                            nc.tensor.matmul(
                                ps_y[:, 0:w],
                                wo_sb[:, d * H + oc * 128:
                                      d * H + (oc + 1) * 128],
                                attn[d][:, lo:lo + w],
                                start=(d == 0), stop=(d == QH - 1))
                        yb = ybp.tile([128, 512], bf16, tag="yb",
                                      name="yb")
                        if alt and oc % 2 == 1:
                            nc.vector.tensor_copy(yb[:, 0:w], ps_y[:, 0:w])
                        else:
                            nc.scalar.copy(yb[:, 0:w], ps_y[:, 0:w])
                        nc.sync.dma_start(
                            out=yT_d[oc * 128:(oc + 1) * 128, lo:lo + w],
                            in_=yb[:, 0:w])
                    units.append(unit)
                return units

            krprev = [None]
            vrawprev = [None]

            def rope_pair(sc, cp_, srcs, cssn_t, kr):
                """srcs: two [128,512] col tensors. cp_<2 -> qpair[cp_];
                cp_==2 -> kr. out = src*cos + swap64(src)*(+-sin)."""
                csl = cssn_t[:, 0:1024]
                snl = cssn_t[:, 1024:2048]
                if cp_ < 2:
                    dst3 = qpair[cp_][:].rearrange(
                        "p (h s) -> p h s", h=2)[:, :,
                                                 sc * 512:(sc + 1) * 512]
                else:
                    dst3 = kr[:].rearrange("p (h s) -> p h s", h=2)
                e1 = eb.tile([128, 1024], f32, tag="e1", name="e1")
                e2 = eb.tile([128, 1024], f32, tag="e2", name="e2")
                for h in range(2):
                    sl = slice(h * 512, (h + 1) * 512)
                    nc.vector.tensor_mul(e1[:, sl], srcs[h][:], csl[:, sl])
                    nc.vector.tensor_mul(e2[0:64, sl], srcs[h][64:128, :],
                                         snl[0:64, sl])
                    nc.vector.tensor_mul(e2[64:128, sl], srcs[h][0:64, :],
                                         snl[64:128, sl])
                e13 = e1[:].rearrange("p (h s) -> p h s", h=2)
                e23 = e2[:].rearrange("p (h s) -> p h s", h=2)
                nc.gpsimd.tensor_add(dst3[:, :, :], e13[:, :, :],
                                     e23[:, :, :])

            def kconv_emit(sc, kr):
                # kconv = kr + (w0k/w1k) * kr_prev-token (w1k in weights)
                for i in range(KVH):
                    r = cw_sb[:, i:i + 1]
                    b = i * 512
                    for jj in range(4):
                        j = 4 * sc + jj
                        lo = b + jj * 128
                        if jj == 0:
                            nc.vector.scalar_tensor_tensor(
                                kct[i][j][:, 1:128], kr[:, b:b + 127], r,
                                kr[:, b + 1:b + 128], mult, add)
                            if sc == 0:
                                nc.vector.tensor_copy(kct[i][j][:, 0:1],
                                                      kr[:, b:b + 1])
                            else:
                                nc.vector.scalar_tensor_tensor(
                                    kct[i][j][:, 0:1],
                                    krprev[0][:, b + 511:b + 512], r,
                                    kr[:, b:b + 1], mult, add)
                        else:
                            nc.vector.scalar_tensor_tensor(
                                kct[i][j][:, 0:128],
                                kr[:, lo - 1:lo + 127], r,
                                kr[:, lo:lo + 128], mult, add)
                krprev[0] = kr

            def vconv_pre(sc, psV):
                """psV: 2 psum tiles [128 d, 512 tok] (one per kv head).
                Copy to SBUF bf16, conv along tokens. Returns vcb."""
                vcb = rl.tile([128, 1024], bf16, tag="vcb", name="vcb")
                vraw = rl.tile([128, 1024], bf16, tag="vraw", name="vraw")
                for i in range(KVH):
                    b = i * 512
                    nc.scalar.copy(vraw[:, b:b + 512], psV[i][:])
                    r = cw_sb[:, 2 + i:3 + i]
                    nc.vector.scalar_tensor_tensor(
                        vcb[:, b + 1:b + 512], vraw[:, b:b + 511], r,
                        vraw[:, b + 1:b + 512], mult, add)
                    if sc == 0:
                        nc.vector.tensor_copy(vcb[:, b:b + 1],
                                              vraw[:, b:b + 1])
                    else:
                        nc.vector.scalar_tensor_tensor(
                            vcb[:, b:b + 1],
                            vrawprev[0][:, b + 511:b + 512], r,
                            vraw[:, b:b + 1], mult, add)
                vrawprev[0] = vraw
                return vcb

            def vconv_transposes(sc, vcb):
                for half in range(2):
                    tps = scp.tile([128, 512], f32, tag="sc", name="vtp")
                    tpb = tps[:].bitcast(bf16)
                    for u in range(4):
                        g = 4 * half + u      # global 128-block 0..7
                        i, sub = g // 4, g % 4
                        nc.tensor.transpose(
                            tpb[:, u * 128:(u + 1) * 128],
                            vcb[:, i * 512 + sub * 128:
                                i * 512 + (sub + 1) * 128],
                            eye_sb[:])
                        nc.vector.tensor_copy(vt[i][4 * sc + sub][:],
                                              tpb[:, u * 128:(u + 1) * 128])

            carry = None

            # ---------------- main software-pipelined loop ----------------
            for t in range(NSC + 1):
                if t >= 1:
                    sa, carry = attention_steps(2 * (t - 1), carry)
                    sb, carry = attention_steps(
                        2 * (t - 1) + 1, carry,
                        last=(2 * (t - 1) + 1 == NT - 1))
                    steps = sa + sb
                else:
                    steps = []
                units = (oproj_units(t - 2, alt=(t == NSC))
                         if t >= 2 else [])
                if t < NSC:
                    fill = steps
                else:
                    fill = steps[:4]
                    rest = steps[4:]
                    for u in range(max(len(units), len(rest))):
                        if u < len(units):
                            fill.append(units[u])
                        if u < len(rest):
                            fill.append(rest[u])
                si = 0

                if t < NSC:
                    if t + 1 < NSC:
                        if t == 0:
                            nhtile = htile1
                        else:
                            nhtile = bht.tile([128, NK * 512], bf16,
                                              tag="htc", name="htc")
                            nc.sync.dma_start(
                                out=nhtile[:].rearrange("p (k s) -> p k s",
                                                        k=NK),
                                in_=hT_d[:, (t + 1) * 512:(t + 2) * 512]
                                .rearrange("(k p) s -> p k s", k=NK))
                        ncssn = load_cssn(t + 1)
                    if t == 0:
                        nc.sync.dma_start(
                            out=wo_sb[:].rearrange("p (d c) -> p d c", d=QH),
                            in_=wo_d[:, :].rearrange("(d p) c -> p d c",
                                                     d=QH))

                    # --- qkv: col-outer, k-inner; one group per full bank.
                    # cols 0..3 = Q heads, 4..5 = K heads, 6..7 = V heads.
                    col_src = [None] * 8
                    nslot = 8 * NK
                    slot = 0
                    kr = rl.tile([128, 1024], bf16, tag="kr", name="kr")
                    vcb = None
                    if True:
                        for col in range(8):
                            ps = mm.tile([128, 512], f32, tag="qkv",
                                         name=f"ps{col}")
                            for k in range(NK):
                                nc.tensor.matmul(
                                    ps[:],
                                    wfs(k, col * 128, (col + 1) * 128),
                                    htile[:, k * 512:(k + 1) * 512],
                                    start=(k == 0), stop=(k == NK - 1))
                                slot += 1
                                want = slot * len(fill) * 3 // (4 * nslot)
                                while si < min(want, len(fill)):
                                    fill[si]()
                                    si += 1
                            col_src[col] = ps
                            if col == 1:
                                rope_pair(t, 0, col_src[0:2], cssn, None)
                            elif col == 3:
                                rope_pair(t, 1, col_src[2:4], cssn, None)
                            elif col == 5:
                                rope_pair(t, 2, col_src[4:6], cssn, kr)
                                kconv_emit(t, kr)
                            elif col == 7:
                                vcb = vconv_pre(t, col_src[6:8])
                    nu = len(units)
                    rest = len(fill) - si
                    for ui in range(max(nu, 1)):
                        if ui < nu:
                            units[ui]()
                        tgt = si + (rest // max(nu, 1)) if ui < nu - 1 \
                            else len(fill)
                        while si < min(tgt, len(fill)):
                            fill[si]()
                            si += 1
                    vconv_transposes(t, vcb)
                    if t + 1 < NSC:
                        htile = nhtile
                        cssn = ncssn
                else:
                    while si < len(fill):
                        fill[si]()
                        si += 1
                    for u in oproj_units(NSC - 1, alt=True):
                        u()

    nc.finalize()
    return nc


def _host_inputs(hidden, W_pack, W_o, conv_k, conv_v):
    """Per-core input maps."""
    bf16 = ml_dtypes.bfloat16
    pos = np.arange(S, dtype=np.float64)
    inv_freq = 1.0 / (THETA ** (np.arange(0, HD, 2, dtype=np.float64) / HD))
    freqs = np.outer(pos, inv_freq)                       # (S, 64)
    cos = np.cos(freqs).T.astype(np.float32)              # (64, S)
    sin = np.sin(freqs).T.astype(np.float32)
    cos = np.concatenate([cos, cos], axis=0)              # (128, S)
    sin = np.concatenate([-sin, sin], axis=0)             # sign folded in
    # per super-chunk: [cos dup x2 head-cols (1024) | +-sin dup (1024)]
    cs = np.broadcast_to(
        cos.reshape(128, NSC, 1, 512),
        (128, NSC, 2, 512)).reshape(128, NSC, 1024)
    sn = np.broadcast_to(
        sin.reshape(128, NSC, 1, 512),
        (128, NSC, 2, 512)).reshape(128, NSC, 1024)
    csn = np.concatenate([cs, sn], axis=2).reshape(128, -1)
    csn = np.ascontiguousarray(csn).astype(bf16)

    kk = np.arange(128)[:, None]
    qq = np.arange(128)[None, :]
    qq2 = np.arange(256)[None, :]

    def double(m):
        return np.concatenate([m, m], axis=1)
    m1024h = double(qq < kk)                       # [128, 256]
    m896 = double(qq2 - kk < 128)                  # [128, 512]
    m0 = double(qq2 >= kk)                         # [128, 512]
    mm128h = double(qq >= kk)                      # [128, 256]
    msk = np.concatenate([m1024h, m896, m0, mm128h],
                         axis=1).astype(bf16)      # [128, 1536]

    one = np.ones((128, 1), bf16)
    eye = np.eye(128).astype(bf16)

    in_maps = []
    for c in range(NCORES):
        b, g = c // TP, c % TP
        hT = np.ascontiguousarray(hidden[b].T).astype(bf16)
        wq = W_pack[:, g * 512:(g + 1) * 512]
        wk = W_pack[:, NH * HD + 2 * g * 128: NH * HD + (2 * g + 2) * 128]
        wv = W_pack[:, NH * HD + NKV * HD + 2 * g * 128:
                    NH * HD + NKV * HD + (2 * g + 2) * 128]
        # fold conv w1 into Wk/Wv (rope is linear; conv comes after rope)
        wk = wk.copy()
        wv = wv.copy()
        for i in range(KVH):
            wk[:, i * 128:(i + 1) * 128] *= conv_k[2 * g + i, 1]
            wv[:, i * 128:(i + 1) * 128] *= conv_v[2 * g + i, 1]
        wpk = np.ascontiguousarray(
            np.concatenate([wq, wk, wv], axis=1)).astype(bf16)
        wo = np.ascontiguousarray(
            W_o[g * 512:(g + 1) * 512, :]).astype(bf16)
        cwv = np.empty(4, np.float32)
        for i in range(KVH):
            cwv[i] = conv_k[2 * g + i, 0] / conv_k[2 * g + i, 1]
            cwv[2 + i] = conv_v[2 * g + i, 0] / conv_v[2 * g + i, 1]
        cw = np.broadcast_to(cwv, (128, 4)).copy()
        in_maps.append({
            "hT": hT, "wpk": wpk, "wo": wo, "csn": csn,
            "cw": cw, "msk": msk, "one": one, "eye": eye,
        })
    return in_maps


def run_cores(in_maps, trace=False, **kw):
    from concourse.bass_utils import run_bass_kernel_spmd
    if "nc" not in _CACHE:
        _CACHE["nc"] = _build_program()
    return run_bass_kernel_spmd(_CACHE["nc"], in_maps, list(range(NCORES)),
                                trace=trace, **kw)


def kernel(hidden, W_pack, W_o, conv_k, conv_v):
    hidden = np.asarray(hidden, np.float32)
    W_pack = np.asarray(W_pack, np.float32)
    W_o = np.asarray(W_o, np.float32)
    conv_k = np.asarray(conv_k, np.float32)
    conv_v = np.asarray(conv_v, np.float32)
    in_maps = _host_inputs(hidden, W_pack, W_o, conv_k, conv_v)
    res = run_cores(in_maps)
    out = np.zeros((B, S, H), np.float32)
    for c in range(NCORES):
        b = c // TP
        out[b] += res.results[c]["yT"].T.astype(np.float32)
    return out
